# revision 58
# baseline (speedup 1.0000x reference)
"""Trainium2 Bass kernel for nn_Encoder_37340445671714 (video ViT encoder).

Sharding: 8 cores = 4 batch elements x 2 sequence halves (788 tokens each).
Each core runs the full 6-layer encoder for its (batch, half):
  - activations kept transposed [feature, token] in SBUF
  - all matmuls bf16 (fp32 PSUM accumulate), residual stream bf16
  - flash-style attention: scores^T per k-tile -> Exp on ScalarE -> AV
    accumulation; V tiles carry a constant ones-block per head (parity
    swapped for odd heads) so the same AV matmul also produces the softmax
    denominator Z; 1/Z = exp(-ln Z) on ScalarE, re-aligned to its head's
    partitions by one f32r matmul against a host anti-diagonal identity
  - LayerNorm stats via ones-matmul partition sums on TensorE
  - per-layer pair AllGather (bf16) exchanges the updated half sequence.
    Keys are processed in a core-local order [own 788 | peer 788] over 13
    k-tiles (one mixed boundary tile assembled at PSUM partition offsets
    0/32/64): the own-key phase (Q/K/V projections interleaved with 6 own
    k-tiles, accumulators spilled to SBUF f32r) runs while the collective
    is in flight; the peer phase re-accumulates and folds the spill back
    in when the gathered x arrives. The peer half of the gather output is
    selected with one partition-id-indexed DMA, so all 8 cores run
    identical code.
Weights are pre-transposed on the host (free) into matmul-ready layouts.
Output is transposed back to natural layout on the PE at the end.
"""

import numpy as np
import ml_dtypes

import concourse.bass as bass
import concourse.tile as tile
from concourse import mybir
from concourse.bass_utils import run_bass_kernel_spmd

F32 = mybir.dt.float32
F32R = mybir.dt.float32r
BF16 = mybir.dt.bfloat16
AF = mybir.ActivationFunctionType
OP = mybir.AluOpType

# problem dims
B, L, C, H, W = 4, 8, 3, 224, 224
PH = PW = 16
D = 512
NH = 8
DK = 64
FF = 2048
NL = 6
NP = (H // PH) * (W // PW)  # 196
S = L * (NP + 1)  # 1576
PD = PH * PW * C  # 768
OWN = S // 2  # 788 tokens per core
LN_EPS = 1e-5

DC = D // 128  # 4 feature subtiles
PDC = PD // 128  # 6
FTC = FF // 128  # 16

# q chunks (the 2 halves of the own-token range)
QC = [(0, 394), (394, 394)]
# Keys are laid out locally as [own 788 | 12 dead | peer 788 | 76 dead] so
# both pair cores run identical code. kT columns: own at 0:788, peer at
# 800:1588; dead columns are zeroed once (scores 0 -> exp 1 -> x zero V).
KCOLS = 1592  # covers the 13th (52-key) tile at 1536:1588, rounded to 8
PEER_BASE = 800
NKT = 13
# (flat kT column, ksz) per k tile; tiles 0..5 pure own, 6 = boundary
# (own tail + peer head at PSUM partition offsets 32/64), 7..12 pure peer
KT_SC = [(i * 128, 128) for i in range(12)] + [(1536, 52)]
# V-tile build pieces: (src, src_col, n, psum_partition_pos)
VP_OWN = [[("own", i * 128, 128, 0)] for i in range(6)]
VP_BOUND = [("own", 768, 20, 0), ("peer", 0, 32, 32), ("peer", 32, 64, 64)]
VP_PEER = [[("peer", 96 + 128 * i, 128, 0)] for i in range(5)] + [
    [("peer", 736, 52, 0)]
]
V_PIECES = VP_OWN + [VP_BOUND] + VP_PEER

N_CORES = 8
REPLICA_GROUPS = [[0, 1], [2, 3], [4, 5], [6, 7]]


def legalize_waits(nc):
    """Split multi-wait instructions into preceding single-wait NoOps.

    The walrus build in this environment rejects instructions carrying more
    than one semaphore wait command.
    """
    n_split = 0
    for f in nc.m.functions:
        for bb in f.blocks:
            insts = list(bb.instructions)
            new_insts = []
            changed = False
            for inst in insts:
                si = inst.sync_info
                if si is not None and len(si.on_wait) > 1:
                    waits = list(si.on_wait)
                    for w in waits[:-1]:
                        nop = mybir.InstNoOp(
                            name=nc.get_next_instruction_name(),
                            engine=inst.engine,
                            ins=[],
                            outs=[],
                        )
                        nop.sync_info = mybir.SyncInfo(on_wait=[w], on_update=[])
                        new_insts.append(nop)
                        n_split += 1
                    inst.sync_info = mybir.SyncInfo(
                        on_wait=[waits[-1]], on_update=list(si.on_update)
                    )
                    changed = True
                new_insts.append(inst)
            if changed:
                bb.instructions = new_insts
    return n_split


def _bcast_ap(ap_1d, parts=128):
    """Partition-broadcast DRAM AP: [n] -> [parts, n] with partition stride 0."""
    return bass.AP(
        tensor=ap_1d.tensor, offset=ap_1d.offset, ap=[[0, parts]] + list(ap_1d.ap)
    )


def build_kernel(passes=1, no_cc=False):
    nc = bass.Bass(
        "TRN2", target_bir_lowering=False, debug=False, num_devices=N_CORES
    )

    # ---- I/O ----
    pat = nc.dram_tensor("pat", [PD, OWN], BF16, kind="ExternalInput").ap()
    addv = nc.dram_tensor("addv", [D, OWN], F32, kind="ExternalInput").ap()
    wembT = nc.dram_tensor("wembT", [PD, D], BF16, kind="ExternalInput").ap()
    wqT = nc.dram_tensor("wqT", [NL, D, D], BF16, kind="ExternalInput").ap()
    wkT = nc.dram_tensor("wkT", [NL, D, D], BF16, kind="ExternalInput").ap()
    wvT = nc.dram_tensor("wvT", [NL, D, D], BF16, kind="ExternalInput").ap()
    woT = nc.dram_tensor("woT", [NL, D, D], BF16, kind="ExternalInput").ap()
    w1T = nc.dram_tensor("w1T", [NL, D, FF], BF16, kind="ExternalInput").ap()
    w2T = nc.dram_tensor("w2T", [NL, FF, D], BF16, kind="ExternalInput").ap()
    # 8 per-layer D-sized params + b1, concatenated host-side into one blob:
    # [bq bk bo b2 g1 be1 g2 be2 b1] = 8*512 + 2048 = 6144 per layer
    bcat = nc.dram_tensor("bcat", [NL, 8 * D + FF], F32, kind="ExternalInput").ap()
    bv = nc.dram_tensor("bv", [NL, D], F32, kind="ExternalInput").ap()
    ident = nc.dram_tensor("ident", [128, 128], BF16, kind="ExternalInput").ap()
    swapid = nc.dram_tensor("swapid", [128, 128], F32R, kind="ExternalInput").ap()
    xout = nc.dram_tensor("xout", [OWN, D], F32, kind="ExternalOutput").ap()

    with tile.TileContext(nc) as tc:
        with (
            tc.tile_pool(name="const", bufs=1) as constp,
            tc.tile_pool(name="wsmall", bufs=2) as wsmall,
            tc.tile_pool(name="wff", bufs=4) as wff,
            tc.tile_pool(name="xp", bufs=2) as xp,
            tc.tile_pool(name="kp", bufs=1) as kp,
            tc.tile_pool(name="vp", bufs=1) as vp,
            tc.tile_pool(name="qo", bufs=1) as qop,
            tc.tile_pool(name="zp", bufs=1) as zp,
            tc.tile_pool(name="zbp", bufs=2) as zbp,
            tc.tile_pool(name="spl", bufs=1) as spl,
            tc.tile_pool(name="big", bufs=1) as bigp,
            tc.tile_pool(name="exps", bufs=3) as expp,
            tc.tile_pool(name="stat", bufs=5) as statp,
            tc.tile_pool(name="rz", bufs=2) as rzp,
            tc.tile_pool(name="bias", bufs=2) as biasp,
            tc.tile_pool(name="psA", bufs=1, space="PSUM") as psA,
            tc.tile_pool(name="psB", bufs=2, space="PSUM") as psB,
            tc.tile_pool(name="dram", bufs=2, space="DRAM") as dramp,
        ):
            P = dict(
                constp=constp, wsmall=wsmall, wff=wff, xp=xp, kp=kp,
                vp=vp, qop=qop, zp=zp, zbp=zbp, spl=spl, bigp=bigp, expp=expp,
                statp=statp, rzp=rzp, biasp=biasp, psA=psA, psB=psB, dramp=dramp,
            )
            dram_in = dict(
                pat=pat, addv=addv, wembT=wembT, wqT=wqT, wkT=wkT, wvT=wvT,
                woT=woT, w1T=w1T, w2T=w2T, bcat=bcat, bv=bv,
                swapid=swapid, ident=ident, xout=xout,
            )
            ones_bf = constp.tile([128, 128], BF16, name="ones_bf")
            nc.vector.memset(ones_bf[:], 1.0)
            ones_r = constp.tile([128, 128], F32R, name="ones_r")
            nc.vector.tensor_scalar_add(ones_r[:], ones_bf[:], 0.0)
            ident_sb = constp.tile([128, 128], BF16, name="ident_sb")
            nc.sync.dma_start(ident_sb[:], ident[:])
            eps_sb = constp.tile([128, 1], F32, name="eps_sb")
            nc.vector.memset(eps_sb[:], LN_EPS)
            swap_sb = constp.tile([128, 128], F32R, name="swap_sb")
            nc.sync.dma_start(swap_sb[:], swapid[:])
            P["ones_bf"] = ones_bf
            P["ones_r"] = ones_r
            P["ident_sb"] = ident_sb
            P["eps_sb"] = eps_sb
            P["swap_sb"] = swap_sb

            # peer index within the pair, materialized once into a register
            # on each engine that issues a dynamic-offset DMA
            P["peer_off"] = {}
            for eng in (nc.sync, nc.gpsimd):
                pid = eng.partition_id()
                P["peer_off"][eng.engine] = eng.snap(
                    (1 - (pid % 2)) * (DC * 128 * OWN),
                    min_val=0,
                    max_val=DC * 128 * OWN,
                )

            # persistent attention K/V buffers:
            #   kT  [128, ct, col]  (K^T, feature-partition, local key order)
            #   v_bf[128, tile, head, 128]  token-partition; per head the
            #   64-wide V block sits at cols 0:64 (even heads) / 64:128
            #   (odd heads), the other 64 cols are a constant ones block
            #   (set once here) that makes the AV matmul also produce Z.
            #   Dead key slots (kT cols 788:800, v rows 20:32 of tile 6)
            #   are zeroed once and never rewritten.
            kT = kp.tile([128, DC, KCOLS], BF16, name="kT")
            nc.vector.memset(kT[:], 0.0)
            v_bf = vp.tile([128, NKT, NH, 128], BF16, name="v_bf")
            vones = (
                v_bf[:]
                .rearrange("p t h e -> p t (h e)")
                .rearrange("p t (j r) -> p t j r", j=4)
            )
            nc.vector.memset(vones[:, :, :, 64:192], 1.0)
            # dead key rows 20:32 of the boundary tile must be all-zero
            # (engine partition offsets are 32-aligned, so zero 0:32 and
            # restore the ones block for the live rows 0:20)
            nc.vector.memset(v_bf[0:32, 6, :, :], 0.0)
            nc.vector.memset(vones[0:20, 6, :, 64:192], 1.0)
            P["kT"] = kT
            P["v_bf"] = v_bf

            x_bf = _embed(nc, P, dram_in)
            for i in range(NL * passes):
                x_bf = _one_layer(
                    nc, P, dram_in, i % NL, x_bf,
                    last=(i == NL * passes - 1),
                    static_peer=(passes > 1),
                    no_cc=no_cc,
                )
            _tail(nc, P, dram_in, x_bf)
    return nc


def _embed(nc, P, dr):
    bigp, zp, wff, xp, psB = P["bigp"], P["zp"], P["wff"], P["xp"], P["psB"]
    pat_sb = bigp.tile([128, FTC, OWN], BF16, tag="h", name="pat_sb")
    nc.sync.dma_start(
        pat_sb[:, :PDC, :], dr["pat"].rearrange("(ko p) t -> p ko t", p=128)
    )
    addv_sb = zp.tile([128, DC, OWN], F32, tag="z", name="addv_sb")
    nc.sync.dma_start(addv_sb[:], dr["addv"].rearrange("(co p) t -> p co t", p=128))
    wemb_sb = wff.tile([128, PDC, D], BF16, tag="wff", name="wemb_sb")
    nc.sync.dma_start(wemb_sb[:], dr["wembT"].rearrange("(ko p) d -> p ko d", p=128))

    x_bf = xp.tile([128, DC, OWN], BF16, tag="x", name="x_emb")
    for dt in range(DC):
        for (q0, qn) in QC:
            ps = psB.tile([128, 2, 512], F32, tag="s", name="ps_emb")
            for kt in range(PDC):
                nc.tensor.matmul(
                    ps[:, 0, :qn],
                    wemb_sb[:, kt, dt * 128 : (dt + 1) * 128],
                    pat_sb[:, kt, q0 : q0 + qn],
                    start=(kt == 0),
                    stop=(kt == PDC - 1),
                )
            nc.vector.tensor_tensor(
                x_bf[:, dt, q0 : q0 + qn],
                ps[:, 0, :qn],
                addv_sb[:, dt, q0 : q0 + qn],
                OP.add,
            )
    return x_bf


def _load_layer_params(nc, P, dr, l):
    biasp, wsmall, wff = P["biasp"], P["wsmall"], P["wff"]
    prm = {}
    bc = biasp.tile([128, 48], F32, tag="bcat", name="bcat_sb")
    nc.sync.dma_start(bc[:], dr["bcat"][l].rearrange("(o p) -> p o", p=128))
    for i, nm in enumerate(["bq", "bk", "bo", "b2", "g1", "be1", "g2", "be2"]):
        prm[nm] = bc[:, i * DC : (i + 1) * DC]
    prm["b1"] = bc[:, 32:48]
    bv_bc = biasp.tile([128, D], BF16, tag="bvb", name="bv_bc")
    nc.gpsimd.dma_start(bv_bc[:], _bcast_ap(dr["bv"][l]))
    prm["bv_bc"] = bv_bc
    # weights go on the Act HWDGE queue so they don't queue behind the
    # x-stage / x_peer DMAs on the SP queue; phase-1-critical ones first
    # wq/wo share one 2-buffer slot (disjoint lifetimes: wq is consumed in
    # phase 1, wo at the O-projection), likewise wk/wv pair up
    for nm, key, tag in [("wq", "wqT", "wqo"), ("wk", "wkT", "wkv"),
                         ("wv", "wvT", "wkv")]:
        t = wsmall.tile([128, DC, D], BF16, tag=tag, name=nm + "_sb")
        nc.scalar.dma_start(t[:], dr[key][l].rearrange("(co p) d -> p co d", p=128))
        prm[nm] = t
    w1_halves, w2_halves = [], []
    for half in range(2):
        w1_sb = wff.tile([128, DC, FF // 2], BF16, tag="wff", name="w1_sb")
        nc.scalar.dma_start(
            w1_sb[:],
            dr["w1T"][l][:, half * (FF // 2) : (half + 1) * (FF // 2)].rearrange(
                "(co p) f -> p co f", p=128
            ),
        )
        w1_halves.append(w1_sb)
    wo_sb = wsmall.tile([128, DC, D], BF16, tag="wqo", name="wo_sb")
    nc.scalar.dma_start(
        wo_sb[:], dr["woT"][l].rearrange("(co p) d -> p co d", p=128)
    )
    prm["wo"] = wo_sb
    for half in range(2):
        w2_sb = wff.tile([128, FTC // 2, D], BF16, tag="wff", name="w2_sb")
        nc.scalar.dma_start(
            w2_sb[:],
            dr["w2T"][l][half * (FF // 2) : (half + 1) * (FF // 2), :].rearrange(
                "(fo p) d -> p fo d", p=128
            ),
        )
        w2_halves.append(w2_sb)
    prm["w1"] = w1_halves
    prm["w2"] = w2_halves
    return prm


def _launch_gather(nc, P, x_bf, static_peer=False, no_cc=False):
    """Stage own x to DRAM (per-ct so it starts as LN2 finishes chunks),
    AllGather across the pair, and DMA the peer half back to SBUF.

    static_peer: read half 1 unconditionally (wrong data on odd cores) —
    used only by the multi-pass timing build, which exceeds the dynamic
    DMA register budget.
    no_cc: timing diagnostic — skip the collective and read back the own
    staged data (wrong results, measures the zero-collective bound)."""
    dramp = P["dramp"]
    xg_in = dramp.tile([DC, 128, OWN], BF16, tag="agi", name="xg_in")
    for ct in range(DC):
        nc.sync.dma_start(xg_in[ct], x_bf[:, ct, :])
    x_peer = P["xp"].tile([128, DC, OWN], BF16, tag="x", name="x_peer")
    if no_cc:
        nc.sync.dma_start(x_peer[:], xg_in[:].rearrange("c p t -> p c t"))
        return x_peer
    xg_out = dramp.tile([2, DC, 128, OWN], BF16, tag="ago", name="xg_out")
    nc.gpsimd.collective_compute(
        "AllGather",
        OP.bypass,
        replica_groups=REPLICA_GROUPS,
        ins=[xg_in[:].opt()],
        outs=[xg_out[:].opt()],
    )
    if static_peer:
        nc.sync.dma_start(x_peer[:], xg_out[1].rearrange("c p t -> p c t"))
        return x_peer
    # peer half only: src offset = peer * (DC*128*OWN) elements
    base = xg_out[0].rearrange("c p t -> p c t")
    src = bass.AP(
        tensor=base.tensor,
        offset=base.offset + P["peer_off"][nc.sync.engine],
        ap=base.ap,
        dep_tracking_offset=base.offset,
    )
    nc.sync.dma_start(x_peer[:], src)
    return x_peer


def _proj_dt(nc, P, w_sb, x_src, dt, out_sb_slices, bias_sb):
    """One dt-slice of a D x D projection: out[:, chunk] per QC chunk."""
    psB = P["psB"]
    for (t0, tn), out_ap in zip(QC, out_sb_slices):
        ps = psB.tile([128, 2, 512], F32, tag="s", name="ps_p")
        for ct in range(DC):
            nc.tensor.matmul(
                ps[:, 0, :tn],
                w_sb[:, ct, dt * 128 : (dt + 1) * 128],
                x_src[:, ct, t0 : t0 + tn],
                start=(ct == 0),
                stop=(ct == DC - 1),
            )
        nc.vector.tensor_scalar_add(out_ap, ps[:, 0, :tn], bias_sb[:, dt : dt + 1])


def _build_v_tile(nc, P, prm, ti, x_own, x_peer):
    """Token-partition V for k-tile ti (bias added, parity-scattered).

    The boundary tile (6) assembles from three pieces at PSUM partition
    offsets 0/32/64 (own tail + the first 96 peer keys)."""
    v_bf, psB = P["v_bf"], P["psB"]
    bvv = prm["bv_bc"][:].rearrange("p (j r) -> p j r", j=4)
    ps = psB.tile([128, 2, 512], F32, tag="s", name="ps_v")
    for (srcn, c0, n, pos) in V_PIECES[ti]:
        x_src = x_own if srcn == "own" else x_peer
        for ct in range(DC):
            nc.tensor.matmul(
                ps[pos : pos + n, 0, :512],
                x_src[:, ct, c0 : c0 + n],
                prm["wv"][:, ct, :],
                start=(ct == 0),
                stop=(ct == DC - 1),
            )
        psv = ps[pos : pos + n, 0, :].rearrange("p (j r) -> p j r", j=4)
        dst = (
            v_bf[pos : pos + n, ti]
            .rearrange("p h e -> p (h e)")
            .rearrange("p (j r) -> p j r", j=4)
        )
        # even heads: V block at cols 0:64 of each head-pair block
        nc.vector.tensor_tensor(
            dst[:, :, 0:64], psv[:, :, 0:64], bvv[pos : pos + n, :, 0:64], OP.add
        )
        # odd heads: V block at cols 192:256
        nc.vector.tensor_tensor(
            dst[:, :, 192:256], psv[:, :, 64:128], bvv[pos : pos + n, :, 64:128],
            OP.add,
        )


def _attn_phase(nc, P, prm, qT, first_half, x_own=None, x_peer=None):
    """Scores+exp+AV over one key phase for all head pairs, with the K/Q/V
    projection work interleaved into the loop so it fills the PE gaps that
    open while ScalarE computes the exps.

    Phase 1 (first_half): k-tiles 0..5 from x_own; Q/K-own dt slices are
    emitted just before the head pair that consumes them; V tiles are
    built one iteration ahead inside head pair 0's loop. p1/p2 spill to
    SBUF f32 at the end of each head pair so the accumulators survive the
    phase boundary while the pair AllGather is still in flight.

    Phase 2: k-tiles 6..12 (boundary + peer); K-peer dt slices interleave
    the same way; after the combine, 1/Z and the oT writes for the head
    pair are emitted immediately."""
    psA, psB, expp = P["psA"], P["psB"], P["expp"]
    kT, v_bf = P["kT"], P["v_bf"]
    sp1, sp2 = P["sp1"], P["sp2"]
    tis = list(range(0, 6)) if first_half else list(range(6, NKT))
    for hpair in range(NH // 2):
        hdt = hpair
        if first_half:
            _proj_dt(
                nc, P, prm["wq"], x_own, hdt,
                [qT[:, hdt, t0 : t0 + tn] for (t0, tn) in QC], prm["bq"],
            )
            _proj_dt(
                nc, P, prm["wk"], x_own, hdt,
                [kT[:, hdt, t0 : t0 + tn] for (t0, tn) in QC], prm["bk"],
            )
        else:
            _proj_dt(
                nc, P, prm["wk"], x_peer, hdt,
                [kT[:, hdt, PEER_BASE + t0 : PEER_BASE + t0 + tn] for (t0, tn) in QC],
                prm["bk"],
            )
        if hpair == 0:
            # V tiles for this phase, before the exp pipeline starts (they
            # share the psB rotation, so inside the loop they would
            # WAR-serialize behind the exp reads)
            for ti in tis:
                _build_v_tile(nc, P, prm, ti, x_own, x_peer)
        p1 = psA.tile([128, 2, 512], F32, tag="o", name="p1")
        p2 = psA.tile([128, 2, 512], F32, tag="zz", name="p2")
        for i, ti in enumerate(tis):
            k0, ksz = KT_SC[ti]
            first, last = (i == 0), (i == len(tis) - 1)
            for sub in range(2):
                hp = sub * 64
                s_ps = psB.tile([128, 2, 512], F32, tag="s", name="s_ps")
                for ci, (q0, qn) in enumerate(QC):
                    nc.tensor.matmul(
                        s_ps[:ksz, ci, :qn],
                        kT[hp : hp + 64, hdt, k0 : k0 + ksz],
                        qT[hp : hp + 64, hdt, q0 : q0 + qn],
                        start=True,
                        stop=True,
                    )
                e_sb = expp.tile([128, 2, 394], BF16, tag="e", name="expS")
                nc.scalar.activation(
                    e_sb[:ksz, :, :], s_ps[:ksz, :, 0:394], AF.Exp, scale=0.125
                )
                dst = p1 if sub == 0 else p2
                for ci, (q0, qn) in enumerate(QC):
                    nc.tensor.matmul(
                        dst[:, ci, :qn],
                        v_bf[:ksz, ti, 2 * hpair + sub, :],
                        e_sb[:ksz, ci, :qn],
                        start=first,
                        stop=last,
                    )
        if first_half:
            for ci in range(2):
                nc.vector.tensor_scalar_add(
                    sp1[:, hpair, ci, :], p1[:, ci, 0:394], 0.0
                )
                nc.vector.tensor_scalar_add(
                    sp2[:, hpair, ci, :], p2[:, ci, 0:394], 0.0
                )
        else:
            for ci in range(2):
                nc.vector.tensor_tensor(
                    sp1[:, hpair, ci, :], sp1[:, hpair, ci, :],
                    p1[:, ci, 0:394], OP.add,
                )
                nc.vector.tensor_tensor(
                    sp2[:, hpair, ci, :], sp2[:, hpair, ci, :],
                    p2[:, ci, 0:394], OP.add,
                )
            _rz_finish(nc, P, hpair)


def _rz_finish(nc, P, hpair):
    """1/Z for one head pair from the combined sp accumulators, then
    oT = o * (1/Z).

    sp1[hpair]: rows 0:64 = o_even, 64:128 = Z_even
    sp2[hpair]: rows 0:64 = Z_odd,  64:128 = o_odd
    1/Z = exp(-ln Z) on ScalarE; the 64-row partition rotation runs on the
    PE (f32r anti-diagonal identity matmul)."""
    psB, rzp = P["psB"], P["rzp"]
    sp1, sp2 = P["sp1"], P["sp2"]
    oT = P["oT"]
    hdt = hpair
    lnmix = rzp.tile([128, 2, 394], F32, tag="rz", name="lnmix")
    nc.scalar.activation(lnmix[0:64, :, :], sp2[0:64, hpair, :, :], AF.Ln)
    nc.scalar.activation(lnmix[64:128, :, :], sp1[64:128, hpair, :, :], AF.Ln)
    rzmix = rzp.tile([128, 2, 394], F32R, tag="rz", name="rzmix")
    nc.scalar.activation(rzmix[:], lnmix[:], AF.Exp, scale=-1.0)
    rsw_ps = psB.tile([128, 2, 512], F32, tag="s", name="rsw_ps")
    for ci in range(2):
        nc.tensor.matmul(
            rsw_ps[:, ci, :394],
            P["swap_sb"][:],
            rzmix[:, ci, :],
            start=True,
            stop=True,
        )
    rzs = rzp.tile([128, 2, 394], F32, tag="rz", name="rzs")
    nc.vector.tensor_scalar_add(rzs[:], rsw_ps[:, :, 0:394], 0.0)
    nc.vector.tensor_tensor(
        oT[0:64, hdt, :].rearrange("p (a b) -> p a b", a=2),
        sp1[0:64, hpair, :, :],
        rzs[0:64, :, :],
        OP.mult,
    )
    nc.vector.tensor_tensor(
        oT[64:128, hdt, :].rearrange("p (a b) -> p a b", a=2),
        sp2[64:128, hpair, :, :],
        rzs[64:128, :, :],
        OP.mult,
    )


def _residual_proj(nc, psB, w_sb, rhs_T, bias_sb, x_bf, z_out):
    """z_out (f32) = W^T proj of rhs_T + bias + x_bf (residual)."""
    for dt in range(DC):
        for (q0, qn) in QC:
            ps = psB.tile([128, 2, 512], F32, tag="s", name="ps_r")
            for ct in range(DC):
                nc.tensor.matmul(
                    ps[:, 0, :qn],
                    w_sb[:, ct, dt * 128 : (dt + 1) * 128],
                    rhs_T[:, ct, q0 : q0 + qn],
                    start=(ct == 0),
                    stop=(ct == DC - 1),
                )
            nc.vector.scalar_tensor_tensor(
                z_out[:, dt, q0 : q0 + qn],
                ps[:, 0, :qn],
                bias_sb[:, dt : dt + 1],
                x_bf[:, dt, q0 : q0 + qn],
                OP.add,
                OP.add,
            )


def _ffn(nc, P, prm, x_bf, z_out):
    psA, psB, bigp = P["psA"], P["psB"], P["bigp"]
    h_bf = bigp.tile([128, FTC, OWN], BF16, tag="h", name="h_bf")
    for half in range(2):
        w1_sb = prm["w1"][half]
        for j in range(FTC // 2):
            ft = half * (FTC // 2) + j
            ps = psB.tile([128, 2, 512], F32, tag="s", name="ps_h")
            for ci, (q0, qn) in enumerate(QC):
                for ct in range(DC):
                    nc.tensor.matmul(
                        ps[:, ci, :qn],
                        w1_sb[:, ct, j * 128 : (j + 1) * 128],
                        x_bf[:, ct, q0 : q0 + qn],
                        start=(ct == 0),
                        stop=(ct == DC - 1),
                    )
            nc.vector.tensor_scalar(
                h_bf[:, ft, :].rearrange("p (a b) -> p a b", a=2),
                ps[:, :, 0:394],
                prm["b1"][:, ft : ft + 1],
                0.0,
                OP.add,
                OP.max,
            )

    for dt in range(DC):
        for ci2, (q0, qn) in enumerate(QC):
            # alternate between the two psA slots (the attention "o"/"zz"
            # slots are idle during the FFN) so accumulation groups
            # double-buffer
            ps2 = psA.tile(
                [128, 2, 512], F32,
                tag=("o" if (dt * 2 + ci2) % 2 == 0 else "zz"), name="ps_f",
            )
            for half in range(2):
                for j in range(FTC // 2):
                    ft = half * (FTC // 2) + j
                    nc.tensor.matmul(
                        ps2[:, 0, :qn],
                        prm["w2"][half][:, j, dt * 128 : (dt + 1) * 128],
                        h_bf[:, ft, q0 : q0 + qn],
                        start=(ft == 0),
                        stop=(ft == FTC - 1),
                    )
            nc.vector.scalar_tensor_tensor(
                z_out[:, dt, q0 : q0 + qn],
                ps2[:, 0, :qn],
                prm["b2"][:, dt : dt + 1],
                x_bf[:, dt, q0 : q0 + qn],
                OP.add,
                OP.add,
            )


def _one_layer(nc, P, dr, l, x_bf, last, static_peer=False, no_cc=False):
    prm = _load_layer_params(nc, P, dr, l)
    x_peer = _launch_gather(nc, P, x_bf, static_peer=static_peer, no_cc=no_cc)

    # spill buffers for the own-half attention partial sums
    sp1 = P["spl"].tile([128, NH // 2, 2, 394], F32, tag="sp1", name=f"sp1_{l}")
    sp2 = P["spl"].tile([128, NH // 2, 2, 394], F32, tag="sp2", name=f"sp2_{l}")
    P["sp1"], P["sp2"] = sp1, sp2
    qT = P["qop"].tile([128, DC, OWN], BF16, tag="qT", name="qT")
    oT = P["qop"].tile([128, DC, OWN], BF16, tag="oT", name="oT")
    P["oT"] = oT

    # ---- phase 1: everything that only needs the own half ----
    _attn_phase(nc, P, prm, qT, first_half=True, x_own=x_bf)
    # ---- phase 2: needs the gathered peer half ----
    _attn_phase(nc, P, prm, qT, first_half=False, x_own=x_bf, x_peer=x_peer)

    z = P["zp"].tile([128, DC, OWN], F32R, tag="z", name=f"z1_{l}")
    _residual_proj(nc, P["psB"], prm["wo"], oT, prm["bo"], x_bf, z)
    x_bf = P["xp"].tile([128, DC, OWN], BF16, tag="x", name=f"x_ln1_{l}")
    _layernorm(nc, P, z, x_bf, prm["g1"], prm["be1"])
    z = P["zp"].tile([128, DC, OWN], F32R, tag="z", name=f"z2_{l}")
    _ffn(nc, P, prm, x_bf, z)
    if last:
        x2 = P["bigp"].tile([128, DC, OWN], F32, tag="h", name="x_final")
    else:
        x2 = P["xp"].tile([128, DC, OWN], BF16, tag="x", name=f"x_ln2_{l}")
    _layernorm(nc, P, z, x2, prm["g2"], prm["be2"])
    return x2


def _tail(nc, P, dr, x_f32):
    psB = P["psB"]
    xout = dr["xout"]
    ident32 = P["constp"].tile([128, 128], F32, name="ident32")
    nc.vector.tensor_scalar_add(ident32[:], P["ident_sb"][:], 0.0)
    for ti in range(7):
        t0 = ti * 128
        tsz = min(128, OWN - t0)
        xo_sb = P["statp"].tile([128, D], F32, tag="st", name="xo_sb")
        for dt in range(DC):
            tp = psB.tile([128, 2, 512], F32, tag="s", name="tp")
            nc.tensor.transpose(
                tp[:tsz, 0, :128], x_f32[:, dt, t0 : t0 + tsz], ident32[:]
            )
            nc.vector.tensor_scalar_add(
                xo_sb[:tsz, dt * 128 : (dt + 1) * 128], tp[:tsz, 0, :128], 0.0
            )
        nc.sync.dma_start(xout[t0 : t0 + tsz, :], xo_sb[:tsz, :])


def _layernorm(nc, P, z, x_out, g_sb, be_sb):
    """Post-LN over features (partition dim) in transposed layout.

    z: [128, DC, OWN] f32r.  Writes x_out = (z - mu) * rstd * g + b.
    Stats are computed with f32r ones-matmuls directly on z; the squared
    tile is produced per-ct into a small rotating buffer."""
    psB, statp, zbp = P["psB"], P["statp"], P["zbp"]
    ones_r = P["ones_r"]
    zf = z[:].bitcast(F32)
    sum_ps = psB.tile([128, 2, 512], F32, tag="s", name="sum_ps")
    for ci, (q0, qn) in enumerate(QC):
        for ct in range(DC):
            nc.tensor.matmul(
                sum_ps[:, ci, :qn],
                ones_r[:],
                z[:, ct, q0 : q0 + qn],
                start=(ct == 0),
                stop=(ct == DC - 1),
            )
    sq_ps = psB.tile([128, 2, 512], F32, tag="s", name="sq_ps")
    for ct in range(DC):
        for ci, (q0, qn) in enumerate(QC):
            sq = zbp.tile([128, 394], F32R, tag="zb", name="sq_r")
            nc.vector.tensor_tensor(
                sq[:], zf[:, ct, q0 : q0 + qn], zf[:, ct, q0 : q0 + qn], OP.mult
            )
            nc.tensor.matmul(
                sq_ps[:, ci, :qn],
                ones_r[:],
                sq[:],
                start=(ct == 0),
                stop=(ct == DC - 1),
            )
    mu = statp.tile([128, 2, 394], F32, tag="st", name="mu")
    nc.vector.tensor_scalar(
        mu[:], sum_ps[:, :, 0:394], 1.0 / D, None, OP.mult, OP.bypass
    )
    musq = statp.tile([128, 2, 394], F32, tag="st", name="musq")
    nc.vector.tensor_tensor(musq[:], mu[:], mu[:], OP.mult)
    var = statp.tile([128, 2, 394], F32, tag="st", name="var")
    nc.vector.scalar_tensor_tensor(
        var[:], sq_ps[:, :, 0:394], 1.0 / D, musq[:], OP.mult, OP.subtract
    )
    # rstd = exp(-0.5 * ln(var + eps))
    lnv = statp.tile([128, 2, 394], F32, tag="st", name="lnv")
    nc.scalar.activation(lnv[:], var[:], AF.Ln, bias=P["eps_sb"][:])
    rstd = statp.tile([128, 2, 394], F32, tag="st", name="rstd")
    nc.scalar.activation(rstd[:], lnv[:], AF.Exp, scale=-0.5)
    mr = statp.tile([128, 2, 394], F32, tag="st", name="mr")
    nc.vector.tensor_tensor(mr[:], mu[:], rstd[:], OP.mult)
    rstd_f = rstd[:].rearrange("p a b -> p (a b)")
    mr_f = mr[:].rearrange("p a b -> p (a b)")
    for ct in range(DC):
        nc.vector.tensor_tensor(z[:, ct, :], zf[:, ct, :], rstd_f[:, :788], OP.mult)
        nc.vector.tensor_tensor(z[:, ct, :], zf[:, ct, :], mr_f[:, :788], OP.subtract)
        nc.scalar.activation(
            x_out[:, ct, :],
            zf[:, ct, :],
            AF.Identity,
            bias=be_sb[:, ct : ct + 1],
            scale=g_sb[:, ct : ct + 1],
        )


_NC_CACHE = None


def _host_prep(inputs):
    """Patchify vid, build per-core inputs, pre-transpose weights (host-side)."""
    bf = ml_dtypes.bfloat16
    vid = np.asarray(inputs["vid"], np.float32)
    x = vid.reshape(B, L, C, H // PH, PH, W // PW, PW)
    x = x.transpose(0, 1, 3, 5, 4, 6, 2).reshape(B, L, NP, PD)

    pos = np.asarray(inputs["pos_emb"], np.float32)[0]  # [L, NP+1, D]
    cls = np.asarray(inputs["cls"], np.float32)[0, :, 0, :]  # [L, D]
    b_emb = np.asarray(inputs["b_embed"], np.float32)  # [D]

    shared = {
        "wembT": np.ascontiguousarray(
            np.asarray(inputs["W_embed"], np.float32).T
        ).astype(bf),
        "wqT": np.ascontiguousarray(
            np.asarray(inputs["Wq"], np.float32).transpose(0, 2, 1)
        ).astype(bf),
        "wkT": np.ascontiguousarray(
            np.asarray(inputs["Wk"], np.float32).transpose(0, 2, 1)
        ).astype(bf),
        "wvT": np.ascontiguousarray(
            np.asarray(inputs["Wv"], np.float32).transpose(0, 2, 1)
        ).astype(bf),
        "woT": np.ascontiguousarray(
            np.asarray(inputs["Wo"], np.float32).transpose(0, 2, 1)
        ).astype(bf),
        "w1T": np.ascontiguousarray(
            np.asarray(inputs["W1"], np.float32).transpose(0, 2, 1)
        ).astype(bf),
        "w2T": np.ascontiguousarray(
            np.asarray(inputs["W2"], np.float32).transpose(0, 2, 1)
        ).astype(bf),
        "bcat": np.concatenate(
            [
                np.asarray(inputs["bq"], np.float32),
                np.asarray(inputs["bk"], np.float32),
                np.asarray(inputs["bo"], np.float32),
                np.asarray(inputs["b2"], np.float32),
                np.asarray(inputs["ln1_g"], np.float32),
                np.asarray(inputs["ln1_b"], np.float32),
                np.asarray(inputs["ln2_g"], np.float32),
                np.asarray(inputs["ln2_b"], np.float32),
                np.asarray(inputs["b1"], np.float32),
            ],
            axis=1,
        ),
        "bv": np.asarray(inputs["bv"], np.float32),
        "ident": np.eye(128, dtype=np.float32).astype(bf),
        "swapid": np.roll(np.eye(128, dtype=np.float32), 64, axis=1),
    }

    in_maps = []
    for c in range(N_CORES):
        b, half = c // 2, c % 2
        f0 = half * (L // 2)
        pat_c = np.zeros((PD, OWN), np.float32)
        addv_c = np.zeros((D, OWN), np.float32)
        for f in range(L // 2):
            fr = f0 + f
            t0 = f * (NP + 1)
            pat_c[:, t0 + 1 : t0 + NP + 1] = x[b, fr].T
            addv_c[:, t0] = pos[fr, 0] + cls[fr]
            addv_c[:, t0 + 1 : t0 + NP + 1] = (
                pos[fr, 1:].T + b_emb[:, None]
            )
        m = {"pat": pat_c.astype(bf), "addv": addv_c}
        m.update(shared)
        in_maps.append(m)
    return in_maps


def kernel(**inputs):
    global _NC_CACHE
    in_maps = _host_prep(inputs)
    if _NC_CACHE is None:
        nc = build_kernel()
        legalize_waits(nc)
        _NC_CACHE = nc
    nc = _NC_CACHE
    res = run_bass_kernel_spmd(nc, in_maps, core_ids=list(range(N_CORES)))
    out = np.zeros((B, S, D), np.float32)
    for c in range(N_CORES):
        b, half = c // 2, c % 2
        out[b, half * OWN : (half + 1) * OWN, :] = res.results[c]["xout"]
    return out
